# revision 72
# baseline (speedup 1.0000x reference)
"""Trainium2 Bass kernel for nn_Composer (gnn_message_passing).

Math per block (DEPTH=2 blocks, same weights):
    tde[t,n]  = tanh( sum_{e,d} W1[t,d,e] * tok[d,n] * dep[e,n] + b1[t] )
    cnz[p,n]  = tanh( sum_{t,d} W2[p,d,t] * tok[d,n] * tde[t,n] + b2[p] )
    tok'[p,i] = base[p] + sum_j wr[j] * (cnz[p,j] - tanh(b2)[p]) * [heads[j]==i]
Final: out = tok' * (heads == 0).

The final mask keeps only "root" rows (heads==0, ~1/S of rows). Dead-code
elimination through the two blocks shrinks the needed work to a handful of
positions per core:
    roots = {i : heads[i]==0}            (the only nonzero output rows)
    K1    = {j : heads[j] in roots}      (block-1 columns that contribute)
    K0    = {j : heads[j] in K1}         (block-0 columns feeding block-1)
The sparse device program computes the bilinears only for those columns;
everything else of the output is exactly zero (tanh(0)=0 structure) or the
constant base vector, both folded in on host exactly as the dense kernel
already did. If the active sets are large (adversarial heads), we fall back
to the dense pipeline below (program built per active-set capacity, cached).

Sparse device pipeline per block (NA = padded active-column count, ~8):
  z1 = tok ⊙ repdep (one DVE op) -> 64 K-tile matmuls (fp8 W1 x bf16)
  -> tanh -> tde; tdeT via PE transpose; bilinear-2 moving operand built
  as NA masked rank-1 outer products on the PE (mask = one-hot
  per-partition tensor_scalar, K=NA keeps tile_position (0,0)-aligned),
  evacuated into fp8 zz (scale 64 to dodge e4m3 subnormals) -> 64
  DoubleRow fp8 matmuls -> tanh -> cnz; scatters are tiny H matmuls with
  the per-column bias (base - colsum*cbg) folded into PSUM via rank-1
  K=1 matmuls from single-partition rows.

Timeline shape (TimelineSim): the 3MB fp8 weight stream bounds the front
(mpack -> W1 -> W2 in 8 chunks; block-0 work overlaps the stream and its
bilinear-2 accumulates chunk-by-chunk as W2 lands); constants needed only
after the W2 gate (repdep1, H01/H1r, bias-fold rows) ride a separate late
DMA behind W2 so they never lengthen the gate; the serial block-1 chain +
output DMA forms the tail. f32 constants ship as bf16 hi/lo halves inside
the one bf16 pack and are reassembled with a single DVE add.
"""

import os
import sys

sys.path.insert(0, "/opt/trn_rl_repo")

import ml_dtypes
import numpy as np

import concourse.bass as bass
import concourse.bacc as bacc
import concourse.mybir as mybir
import concourse.tile as tile
from concourse.bass_utils import run_bass_kernel_spmd

B, S, D, E, T = 16, 128, 128, 64, 128
V_TOK, V_DEP = 100000, 64
DEPTH = 2
NCORES = 8
BL = B // NCORES  # local batches per core
N = BL * S        # positions per core
F32 = mybir.dt.float32
I32 = mybir.dt.int32
BF16 = mybir.dt.bfloat16
F8 = mybir.dt.float8e4
WSCALE = 8.0  # weights shipped as fp8e4 * WSCALE; undone via activation scale
ZSCALE = 64.0  # fp8 zz scaling (avoids e4m3 subnormals); undone at tanh

LAST_EXEC_TIME_NS = None

_PROG_CACHE = {}


def _pad_cap(n, lo=4):
    """Pad a live-set size up to a small friendly capacity."""
    c = lo
    while c < n:
        c *= 2
    return c


# ---------------------------------------------------------------------------
# sparse (dead-code-eliminated) program
# ---------------------------------------------------------------------------

def build_sparse_program(NA0, NA1, NR):
    MV = BF16
    nc = bacc.Bacc("TRN2", target_bir_lowering=False, debug=False)
    w1t = nc.dram_tensor("W1t", [128, (E * D // 128) * T], F8, kind="ExternalInput")
    w2t = nc.dram_tensor("W2t", [128, (T * D // 128) * D], F8, kind="ExternalInput")
    # f32 pack: b1 | b2 | b01 [128,NA1] | b01T block [NA1 rows x 128] |
    #           bRT block [NR rows x 128]
    FC_B1 = 0
    FC_B2 = 1
    NAmax = max(NA0, NA1)
    FC_OH = 2
    FC_TOT = FC_OH + NAmax
    # bf16 pack: ident [128,128] | tok0a [128,NA0] | tokT0 block [NA0 x 128] |
    #            repdep0 [128, 64*NA0] | repdep1 [128, 64*NA1] |
    #            H01 block [NA0 x NA1] | H1r block [NA1 x NR]
    MC_TOKT0 = 0
    MC_RD0 = MC_TOKT0 + 128
    MC_FHI = MC_RD0 + E * NA0    # f32 constants, high bf16 halves
    MC_FLO = MC_FHI + FC_TOT     # f32 constants, low bf16 halves
    MC_TOT = MC_FLO + FC_TOT
    # late pack (after W2), two DMAs: small scatter/fold constants first
    # (earliest consumer), then the bulky repdep1 + bRT halves
    LC_H01 = 0                   # H01 block [NA0 rows x NA1]
    LC_H1R = LC_H01 + NA1        # H1r block [NA1 rows x NR]
    LC_BROW = LC_H1R + NR        # base row [1, 128]
    LC_CGROW = LC_BROW + 128     # tanh(b2) row [1, 128]
    LC_ONES = LC_CGROW + 128     # ones row [1, NAmax]
    LC_NC1 = LC_ONES + NAmax     # -colsum(H01) row [1, NA1]
    LCA = LC_NC1 + NA1           # end of the small group
    LC_RD1 = LCA
    LC_BHI = LC_RD1 + E * NA1    # bRT f32 high bf16 halves [128 cols]
    LC_BLO = LC_BHI + 128        # bRT f32 low bf16 halves
    LC_TOT = LC_BLO + 128
    mpack = nc.dram_tensor("mpack", [128, MC_TOT], MV, kind="ExternalInput")
    lateh = nc.dram_tensor("latep", [128, LC_TOT], MV, kind="ExternalInput")
    outh = nc.dram_tensor("out", [NR, 128], F32, kind="ExternalOutput")

    with tile.TileContext(nc) as tc:
        with (
            tc.tile_pool(name="const", bufs=1) as cpool,
            tc.tile_pool(name="wres", bufs=1) as wpool,
            tc.tile_pool(name="work", bufs=2) as work,
            tc.tile_pool(name="rows", bufs=2) as rows,
            tc.tile_pool(name="mrows", bufs=2 * max(NA0, NA1)) as mrows,
            tc.tile_pool(name="zzp", bufs=2) as zzp,
            tc.tile_pool(name="psacc", bufs=2, space="PSUM") as psacc,
            tc.tile_pool(name="psout", bufs=2, space="PSUM") as psout,
        ):
            pstr = psacc
            pssc = psout
            # ---- input DMAs (small packs first, then weights)
            mp = cpool.tile([128, MC_TOT], MV, tag="mpack")
            nc.sync.dma_start(mp[:], mpack[:])
            fp = cpool.tile([128, FC_TOT], F32, tag="fpack")
            nc.vector.tensor_tensor(
                out=fp[:],
                in0=mp[:, MC_FHI : MC_FHI + FC_TOT],
                in1=mp[:, MC_FLO : MC_FLO + FC_TOT],
                op=mybir.AluOpType.add,
            )
            w1 = wpool.tile([128, (E * D // 128) * T], F8, tag="w1")
            nc.sync.dma_start(w1[:], w1t[:])
            w2 = wpool.tile([128, (T * D // 128) * D], F8, tag="w2")
            W2CH = 8
            w2csz = (T * D // 128) * D // W2CH
            for ch in range(W2CH):
                sl = slice(ch * w2csz, (ch + 1) * w2csz)
                nc.sync.dma_start(w2[:, sl], w2t[:, sl])
            lp = cpool.tile([128, LC_TOT], MV, tag="latep")
            nc.sync.dma_start(lp[:, 0:LCA], lateh[:, 0:LCA])
            nc.sync.dma_start(lp[:, LCA:LC_TOT], lateh[:, LCA:LC_TOT])

            # identity built on device: is_equal(iota_f, iota_p), all
            # values non-negative (negative channel_multiplier wraps on hw)
            idxf = work.tile([128, 128], I32, tag="idxf")
            nc.gpsimd.iota(idxf[:], pattern=[[1, 128]], channel_multiplier=0)
            idxp = work.tile([128, 1], I32, tag="idxp")
            nc.gpsimd.iota(idxp[:], pattern=[[0, 1]], channel_multiplier=1)
            identt = cpool.tile([128, 128], MV, tag="identt")
            nc.vector.tensor_tensor(
                out=identt[:],
                in0=idxf[:],
                in1=idxp[:, 0:1].to_broadcast((128, 128)),
                op=mybir.AluOpType.is_equal,
            )
            identb = identt[:]
            b1c = fp[:, FC_B1 : FC_B1 + 1]
            b2c = fp[:, FC_B2 : FC_B2 + 1]
            onehot = fp[:, FC_OH : FC_OH + NAmax]
            # bRT rides the late pack (first needed at the final add)
            fp2 = cpool.tile([128, 128], F32, tag="fp2")
            nc.vector.tensor_tensor(
                out=fp2[:],
                in0=lp[:, LC_BHI : LC_BHI + 128],
                in1=lp[:, LC_BLO : LC_BLO + 128],
                op=mybir.AluOpType.add,
            )
            bRT = fp2[0:NR, :]
            tokT0 = mp[0:NA0, MC_TOKT0 : MC_TOKT0 + 128]
            # derive the column orientation on device (off the critical path)
            psk = pstr.tile([128, 128], MV, tag="tr", name="tr")[:, 0:NA0]
            nc.tensor.transpose(psk, tokT0, identb[0:NA0, 0:NA0])
            tok0a = work.tile([128, NA0], MV, tag="tok0a")
            nc.vector.tensor_copy(tok0a[:], psk)
            repdep0 = mp[:, MC_RD0 : MC_RD0 + E * NA0]
            repdep1 = lp[:, LC_RD1 : LC_RD1 + E * NA1]
            H01 = lp[0:NA0, LC_H01 : LC_H01 + NA1]
            H1r = lp[0:NA1, LC_H1R : LC_H1R + NR]
            brow = lp[0:1, LC_BROW : LC_BROW + 128]
            cgrow = lp[0:1, LC_CGROW : LC_CGROW + 128]
            onesrow = lp[0:1, LC_ONES : LC_ONES + NAmax]
            nc1row = lp[0:1, LC_NC1 : LC_NC1 + NA1]

            def bilinear1(tok_c, repdep_c, NA, scale=1.0 / WSCALE, rep_f8=None):
                """z1 = tok ⊙ repdep; ps1 = sum_i W1_i @ z1_i; returns tde.
                z1 is built in halves so the first matmuls start early."""
                z1 = work.tile([128, E * NA], MV, tag="z1")
                rep = rep_f8 if repdep_c is None else repdep_c
                ps1 = psacc.tile([128, NAmax], F32, tag="acc", name="acc")[:, 0:NA]
                EH = E // 2
                for h in range(2):
                    zv = z1[:, h * EH * NA : (h + 1) * EH * NA]
                    nc.vector.tensor_tensor(
                        out=zv.rearrange("p (i c) -> p i c", i=EH),
                        in0=tok_c[:, None, :].to_broadcast((128, EH, NA)),
                        in1=rep[:, h * EH * NA : (h + 1) * EH * NA].rearrange(
                            "p (i c) -> p i c", i=EH
                        ),
                        op=mybir.AluOpType.mult,
                    )
                for i in range(E):
                    nc.tensor.matmul(
                        ps1[:],
                        lhsT=w1[:, i * 128 : (i + 1) * 128],
                        rhs=z1[:, i * NA : (i + 1) * NA],
                        start=(i == 0),
                        stop=(i == E - 1),
                    )
                tde = work.tile([128, NA], MV, tag="tde")
                nc.scalar.activation(
                    tde[:],
                    ps1[:],
                    mybir.ActivationFunctionType.Tanh,
                    bias=b1c,
                    scale=scale,
                )
                return tde

            def transpose_rows(mat, NA, tag, eng=None):
                """[128, NA] sbuf -> [NA, 128] sbuf via PE transpose."""
                psT = pstr.tile([128, 128], MV, tag="tr", name="tr")[0:NA, :]
                nc.tensor.transpose(psT, mat[:], identb)
                matT = rows.tile([128, 128], MV, tag="rT", name="rT")[0:NA, :]
                (eng or nc.vector).tensor_copy(matT, psT)
                return matT

            def bilinear2(tokT_c, tde_c, NA, blk):
                """zz[d,(t,c)] = tok[d,c]*tde[t,c] via masked rank-1 outer
                matmuls; ps2 = sum_t W2_t @ zz_t; returns cnz (bf16)."""
                tdeT = transpose_rows(tde_c, NA, f"tde{blk}")
                zz = zzp.tile([128, T * NA], F8, tag="zz")
                GRP = 4  # outer tiles per psum tile
                for g in range(NA // GRP):
                    pso = psout.tile([128, GRP * 128], F32, tag="pso", name="pso")
                    for k in range(GRP):
                        c = g * GRP + k
                        mrow = mrows.tile([NA, 128], MV, tag="mrow")
                        # mask to row c via one-hot per-partition scalar
                        nc.vector.tensor_scalar(
                            out=mrow[:],
                            in0=tokT_c[:],
                            scalar1=onehot[0:NA, c : c + 1],
                            scalar2=None,
                            op0=mybir.AluOpType.mult,
                        )
                        nc.tensor.matmul(
                            pso[:, k * 128 : (k + 1) * 128],
                            lhsT=mrow[:],
                            rhs=tdeT[:],
                            start=True,
                            stop=True,
                            skip_group_check=True,
                        )
                    # evacuate bank -> zz[d, t*NA + c] (alternate ACT/DVE)
                    zv = zz[:].rearrange("p (t c) -> p t c", c=NA)[
                        :, :, g * GRP : (g + 1) * GRP
                    ]
                    pv = pso[:].rearrange("p (c t) -> p c t", c=GRP)
                    if g % 2 == 0:
                        nc.scalar.activation(
                            zv.rearrange("p t c -> p c t"),
                            pv,
                            mybir.ActivationFunctionType.Identity,
                            scale=ZSCALE,
                        )
                    else:
                        nc.vector.tensor_scalar(
                            out=zv.rearrange("p t c -> p c t"),
                            in0=pv,
                            scalar1=ZSCALE,
                            scalar2=None,
                            op0=mybir.AluOpType.mult,
                        )
                ps2 = psacc.tile([128, NAmax], F32, tag="acc", name="acc")[:, 0:NA]
                for t2 in range(T // 2):
                    nc.tensor.matmul(
                        ps2[:],
                        lhsT=w2[:, t2 * 256 : (t2 + 1) * 256].rearrange(
                            "p (two m) -> p two m", two=2
                        ),
                        rhs=zz[:, t2 * 2 * NA : (t2 + 1) * 2 * NA].rearrange(
                            "p (two c) -> p two c", two=2
                        ),
                        start=(t2 == 0),
                        stop=(t2 == T // 2 - 1),
                        perf_mode=mybir.MatmulPerfMode.DoubleRow,
                    )
                cnz = work.tile([128, NA], MV, tag="cnz")
                nc.scalar.activation(
                    cnz[:],
                    ps2[:],
                    mybir.ActivationFunctionType.Tanh,
                    bias=b2c,
                    scale=1.0 / (WSCALE * ZSCALE),
                )
                return cnz

            # ================= block 0 =================
            tde0 = bilinear1(tok0a, repdep0, NA0)
            cnz0 = bilinear2(tokT0, tde0, NA0, 0)
            cnzT0 = transpose_rows(cnz0, NA0, "c0")
            # scatter to block-1 active columns, both orientations; the
            # per-column bias b01 = base - colsum*cbg lands in PSUM via two
            # rank-1 K=1 matmuls, so tok1 is consumed straight from PSUM
            # bias folds first: they depend only on the late pack, so the
            # PE runs them while waiting for cnzT0's evacuation
            psA = pssc.tile([128, 128], F32, tag="sc", name="sc")[:, 0:NA1]
            nc.tensor.matmul(
                psA, lhsT=brow, rhs=onesrow[:, 0:NA1], start=True, stop=False
            )
            nc.tensor.matmul(psA, lhsT=cgrow, rhs=nc1row, start=False, stop=False)
            psB = pssc.tile([128, 128], F32, tag="sc", name="sc")[0:NA1, :]
            nc.tensor.matmul(
                psB, lhsT=onesrow[:, 0:NA1], rhs=brow, start=True, stop=False
            )
            nc.tensor.matmul(psB, lhsT=nc1row, rhs=cgrow, start=False, stop=False)
            nc.tensor.matmul(psA, lhsT=cnzT0[:], rhs=H01, start=False, stop=True)
            nc.tensor.matmul(psB, lhsT=H01, rhs=cnzT0[:], start=False, stop=True)

            tok1aT = rows.tile([NA1, 128], MV, tag="tok1aT")
            nc.scalar.activation(
                tok1aT[:], psB, mybir.ActivationFunctionType.Identity
            )

            # ================= block 1 =================
            # z1b reads tok1 straight from PSUM (saves the copy stage)
            tde1 = bilinear1(psA, repdep1, NA1)
            cnz1 = bilinear2(tok1aT, tde1, NA1, 1)
            cnzT1 = transpose_rows(cnz1, NA1, "c1")
            psR = pssc.tile([128, 128], F32, tag="sc", name="sc")[0:NR, :]
            nc.tensor.matmul(psR, lhsT=H1r, rhs=cnzT1[:], start=True, stop=True)
            osb = work.tile([NR, 128], F32, tag="osb")
            nc.vector.tensor_tensor(
                out=osb[:], in0=psR, in1=bRT, op=mybir.AluOpType.add
            )
            nc.sync.dma_start(outh[:], osb[:])
    nc.compile()
    return nc


def _sparse_sets(heads_c):
    """Active sets for one core's batches. heads_c: [BL, S] int."""
    roots, k1, k0 = [], [], []
    for b in range(BL):
        h = heads_c[b]
        rb = np.where(h == 0)[0]
        k1b = np.where(np.isin(h, rb))[0]
        k0b = np.where(np.isin(h, k1b))[0]
        roots += [(b, int(i)) for i in rb]
        k1 += [(b, int(j)) for j in k1b]
        k0 += [(b, int(j)) for j in k0b]
    return roots, k1, k0


def kernel_sparse(token_table, dep_table, W1, b1, W2, b2, wr, br,
                  tokens, dep_types, dep_heads, core_sets, NA0, NA1, NR):
    global LAST_EXEC_TIME_NS
    key = ("sparse", NA0, NA1, NR)
    if key not in _PROG_CACHE:
        _PROG_CACHE[key] = build_sparse_program(NA0, NA1, NR)
    nc = _PROG_CACHE[key]

    # weight-layout prep (host): K-tiled stationary operands (as dense path)
    W1f = W1.transpose(2, 1, 0).reshape(E * D, T)  # [(e,d), t]
    W1t = (np.ascontiguousarray(
        W1f.reshape(E * D // 128, 128, T).transpose(1, 0, 2).reshape(128, -1)
    ) * WSCALE).astype(ml_dtypes.float8_e4m3)
    W2f = W2.transpose(2, 1, 0).reshape(T * D, D)  # [(t,d), p]
    W2t = (np.ascontiguousarray(
        W2f.reshape(T * D // 128, 128, D).transpose(1, 0, 2).reshape(128, -1)
    ) * WSCALE).astype(ml_dtypes.float8_e4m3)
    c_bg = np.tanh(b2)
    base = (np.sum(wr) * c_bg + br).astype(np.float32)

    NAmax = max(NA0, NA1)
    FC_OH = 2
    FC_TOT = FC_OH + NAmax
    MC_TOKT0 = 0
    MC_RD0 = MC_TOKT0 + 128
    MC_FHI = MC_RD0 + E * NA0
    MC_FLO = MC_FHI + FC_TOT
    MC_TOT = MC_FLO + FC_TOT
    LC_H01 = 0
    LC_H1R = LC_H01 + NA1
    LC_BROW = LC_H1R + NR
    LC_CGROW = LC_BROW + 128
    LC_ONES = LC_CGROW + 128
    LC_NC1 = LC_ONES + NAmax
    LCA = LC_NC1 + NA1
    LC_RD1 = LCA
    LC_BHI = LC_RD1 + E * NA1
    LC_BLO = LC_BHI + 128
    LC_TOT = LC_BLO + 128

    in_maps = []
    for c in range(NCORES):
        bs = slice(c * BL, (c + 1) * BL)
        toks_c = tokens[bs]
        deps_c = dep_types[bs]
        heads_c = dep_heads[bs]
        roots, k1, k0 = core_sets[c]
        n0, n1, nr = len(k0), len(k1), len(roots)

        fpack = np.zeros((128, FC_TOT), dtype=np.float32)
        fpack[:, 0] = b1
        fpack[:, 1] = b2
        fpack[0:NAmax, FC_OH : FC_OH + NAmax] = np.eye(NAmax, dtype=np.float32)
        mpack = np.zeros((128, MC_TOT), dtype=np.float32)

        # gathered embeddings for active columns
        if n0:
            t0 = token_table[[toks_c[b, j] for (b, j) in k0]]        # [n0, D]
            d0 = dep_table[[deps_c[b, j] for (b, j) in k0]]          # [n0, E]
            mpack[0:n0, MC_TOKT0 : MC_TOKT0 + 128] = t0
            mpack[:, MC_RD0 : MC_RD0 + E * NA0] = np.broadcast_to(
                np.pad(d0.T, ((0, 0), (0, NA0 - n0))).reshape(1, E * NA0),
                (128, E * NA0),
            )
        latep = np.zeros((128, LC_TOT), dtype=np.float32)
        if n1:
            d1 = dep_table[[deps_c[b, j] for (b, j) in k1]]          # [n1, E]
            latep[:, LC_RD1 : LC_RD1 + E * NA1] = np.broadcast_to(
                np.pad(d1.T, ((0, 0), (0, NA1 - n1))).reshape(1, E * NA1),
                (128, E * NA1),
            )

        # scatter matrices with wr folded in
        H01 = np.zeros((NA0, NA1), dtype=np.float32)
        for c0, (b0, j0) in enumerate(k0):
            for c1, (b1_, j1) in enumerate(k1):
                if b0 == b1_ and heads_c[b0, j0] == j1:
                    H01[c0, c1] = wr[j0]
        H1r = np.zeros((NA1, NR), dtype=np.float32)
        for c1, (b1_, j1) in enumerate(k1):
            for r, (br_, ir) in enumerate(roots):
                if b1_ == br_ and heads_c[b1_, j1] == ir:
                    H1r[c1, r] = wr[j1]
        latep[0:NA0, LC_H01 : LC_H01 + NA1] = H01
        latep[0:NA1, LC_H1R : LC_H1R + NR] = H1r

        # bias rows for the rank-1 PSUM folds: the scatters use cnz (not
        # cnz - cbg), so bias = base - colsum*cbg, split into two rank-1
        # terms base⊗ones + cbg⊗(-colsum)
        colsum1 = H01.sum(axis=0)                                   # [NA1]
        colsumR = H1r.sum(axis=0)                                   # [NR]
        latep[0, LC_BROW : LC_BROW + 128] = base
        latep[0, LC_CGROW : LC_CGROW + 128] = c_bg
        latep[0, LC_ONES : LC_ONES + NAmax] = 1.0
        latep[0, LC_NC1 : LC_NC1 + NA1] = -colsum1
        # carry the f32 pack as bf16 hi/lo halves inside mpack (one DMA)
        fhi = fpack.astype(ml_dtypes.bfloat16).astype(np.float32)
        flo = fpack - fhi
        mpack[:, MC_FHI : MC_FHI + FC_TOT] = fhi
        mpack[:, MC_FLO : MC_FLO + FC_TOT] = flo
        # final-row bias (base - colsumR*cbg) as hi/lo halves in the late pack
        bRTf = np.zeros((128, 128), dtype=np.float32)
        bRTf[0:NR] = base[None, :] - np.outer(colsumR, c_bg)
        bhi = bRTf.astype(ml_dtypes.bfloat16).astype(np.float32)
        latep[:, LC_BHI : LC_BHI + 128] = bhi
        latep[:, LC_BLO : LC_BLO + 128] = bRTf - bhi

        in_maps.append(
            {
                "W1t": W1t,
                "W2t": W2t,
                "mpack": np.ascontiguousarray(mpack.astype(ml_dtypes.bfloat16)),
                "latep": np.ascontiguousarray(latep.astype(ml_dtypes.bfloat16)),
            }
        )

    trace = bool(int(os.environ.get("KERNEL_TRACE", "0")))
    res = run_bass_kernel_spmd(nc, in_maps, list(range(NCORES)), trace=trace)
    LAST_EXEC_TIME_NS = res.exec_time_ns
    out = np.zeros((B, S, D), dtype=np.float32)
    for c in range(NCORES):
        roots = core_sets[c][0]
        dev = res.results[c]["out"]                                 # [NR, 128]
        for r, (b, i) in enumerate(roots):
            out[c * BL + b, i, :] = dev[r]
    return out


# ---------------------------------------------------------------------------
# dense fallback (original pipeline)
# ---------------------------------------------------------------------------

CH_Z = 16  # dep-rep chunk size (j-tiles per chunk; 64 z-tiles total)
CH_X = 16  # tde-rep chunk size (128 x-tiles total)

REP_ROUTES = [
    ["e", "d", "e", "p", "d", "e", "p", "d"],  # block 0: DMA busy with W2
    ["e", "d", "e", "p", "d", "d", "p", "d"],  # block 1: DMA bus is free
]

C_IDENT = 0          # [0,128)   identity
C_B1 = 128           # b1
C_B2 = 129           # b2
C_CBG = 130          # tanh(b2)
C_BASE = 131         # sum(wr)*tanh(b2)+br
C_BASEB = 132        # [132,132+N): outer(mask_b, base) per batch, [n,d] rows
C_TOT = 132 + N


def build_program():
    MV = BF16
    nc = bacc.Bacc("TRN2", target_bir_lowering=False, debug=False)
    w1t = nc.dram_tensor("W1t", [128, (E * D // 128) * T], F8, kind="ExternalInput")
    w2t = nc.dram_tensor("W2t", [128, (T * D // 128) * D], F8, kind="ExternalInput")
    cpackh = nc.dram_tensor("cpack", [128, C_TOT], F32, kind="ExternalInput")
    hh = nc.dram_tensor("Hpack", [128, DEPTH * BL * 128], MV, kind="ExternalInput")
    tok0h = nc.dram_tensor("tok0", [128, N], MV, kind="ExternalInput")
    deph = nc.dram_tensor("dep_flat", [1, E * N], MV, kind="ExternalInput")
    depch = nc.dram_tensor("dep_cmp", [E, N], MV, kind="ExternalInput")
    outh = nc.dram_tensor("out", [BL, S, D], F32, kind="ExternalOutput")

    NZ = E // CH_Z
    NX = T // CH_X

    with tile.TileContext(nc) as tc:
        with (
            tc.tile_pool(name="const", bufs=1) as cpool,
            tc.tile_pool(name="wres", bufs=1) as wpool,
            tc.tile_pool(name="zc", bufs=4) as zpool,
            tc.tile_pool(name="rept", bufs=4) as rtpool,
            tc.tile_pool(name="reprow", bufs=2) as rowpool,
            tc.tile_pool(name="xc", bufs=4) as xpool,
            tc.tile_pool(name="work", bufs=2) as work,
            tc.tile_pool(name="psmm", bufs=2, space="PSUM") as pspool,
            tc.tile_pool(name="pssm", bufs=2, space="PSUM") as pssm,
            tc.tile_pool(name="psrep", bufs=2, space="PSUM") as psrep,
            tc.tile_pool(name="dramsc", bufs=2, space="DRAM") as dpool,
        ):
            # ---- packed constants + initial tok first (head of pipeline)
            cpack = cpool.tile([128, C_TOT], F32)
            nc.sync.dma_start(cpack[:], cpackh[:])
            tok0 = cpool.tile([128, N], MV, tag="tok0")
            nc.sync.dma_start(tok0[:], tok0h[:])
            ident = cpack[:, C_IDENT : C_IDENT + 128]
            b1c = cpack[:, C_B1 : C_B1 + 1]
            b2c = cpack[:, C_B2 : C_B2 + 1]
            cbg = cpack[:, C_CBG : C_CBG + 1]
            basec = cpack[:, C_BASE : C_BASE + 1]
            baseB = cpack[:, C_BASEB : C_BASEB + N]

            ident_mv = cpool.tile([128, 128], MV, tag="identmv")
            nc.vector.tensor_copy(ident_mv[:], ident)

            depc = cpool.tile([E, N], MV, tag="depc")
            nc.sync.dma_start(depc[:], depch[:])
            repdep = wpool.tile([128, E * N], MV, tag="repdep")
            w1 = wpool.tile([128, (E * D // 128) * T], F8, tag="w1")
            for ch in range(4):
                sl = slice(ch * CH_Z * N, (ch + 1) * CH_Z * N)
                if ch < 2:
                    nc.sync.dma_start(
                        repdep[:, sl], deph[0:1, sl].to_broadcast((128, CH_Z * N))
                    )
                else:
                    for sub in range(CH_Z // 2):
                        psb = psrep.tile([128, 2 * N], F32, tag="psrep")
                        for jj in range(2):
                            e = ch * CH_Z + sub * 2 + jj
                            nc.tensor.matmul(
                                psb[:, jj * N : (jj + 1) * N],
                                lhsT=ident_mv[0:E, e : e + 1].to_broadcast((E, 128)),
                                rhs=depc[:, :],
                                start=True,
                                stop=True,
                                skip_group_check=True,
                            )
                        lo = (ch * CH_Z + sub * 2) * N
                        nc.scalar.activation(
                            repdep[:, lo : lo + 2 * N],
                            psb[:],
                            mybir.ActivationFunctionType.Identity,
                        )
                slw = slice(ch * 16 * 128, (ch + 1) * 16 * 128)
                nc.sync.dma_start(w1[:, slw], w1t[:, slw])
            Hp = cpool.tile([128, DEPTH * BL * 128], MV, tag="Hp")
            nc.sync.dma_start(Hp[:], hh[:])
            w2 = wpool.tile([128, (T * D // 128) * D], F8, tag="w2")
            for ch in range(8):
                sl = slice(ch * 16 * 128, (ch + 1) * 16 * 128)
                nc.sync.dma_start(w2[:, sl], w2t[:, sl])

            tok_cur = tok0
            for blk in range(DEPTH):
                # ================= bilinear 1 =================
                if blk == 0:
                    halves = [(0, N)]
                    ps1a = pspool.tile([128, N], F32, tag="psmm")
                    ps1s = [ps1a]
                else:
                    halves = [(0, S), (S, S)]
                    ps1a = pspool.tile([128, N], F32, tag="psmm")
                    ps1b = pspool.tile([128, N], F32, tag="psmm")
                    ps1s = [ps1a, ps1b]
                for ch in range(NZ):
                    zc = zpool.tile([128, CH_Z * N], MV, tag="zc")
                    for hi, (h0, hw) in enumerate(halves):
                        nc.vector.tensor_tensor(
                            out=zc[:].rearrange("p (c n) -> p c n", c=CH_Z)[
                                :, :, h0 : h0 + hw
                            ],
                            in0=tok_cur[:, None, h0 : h0 + hw].to_broadcast(
                                (128, CH_Z, hw)
                            ),
                            in1=repdep[
                                :, ch * CH_Z * N : (ch + 1) * CH_Z * N
                            ].rearrange("p (c n) -> p c n", c=CH_Z)[
                                :, :, h0 : h0 + hw
                            ],
                            op=mybir.AluOpType.mult,
                        )
                        for jl in range(CH_Z):
                            i = ch * CH_Z + jl
                            nc.tensor.matmul(
                                ps1s[hi][:, h0 : h0 + hw],
                                lhsT=w1[:, i * 128 : (i + 1) * 128],
                                rhs=zc[:, jl * N + h0 : jl * N + h0 + hw],
                                start=(i == 0),
                                stop=(i == E - 1),
                            )
                tde = work.tile([128, N], MV, tag="tde")
                for hi, (h0, hw) in enumerate(halves):
                    nc.scalar.activation(
                        tde[:, h0 : h0 + hw],
                        ps1s[hi][:, h0 : h0 + hw],
                        mybir.ActivationFunctionType.Tanh,
                        bias=b1c,
                        scale=1.0 / WSCALE,
                    )

                route = REP_ROUTES[blk]
                scr = dpool.tile([128, N], MV, tag="scr")
                if "d" in route or "p" in route:
                    nc.sync.dma_start(scr[:], tde[:])
                scr_flat = scr[:].rearrange("j n -> (j n)")

                # ================= bilinear 2 =================
                ps2 = pspool.tile([128, N], F32, tag="psmm")

                def mm2(j, xcbuf, jb):
                    nc.tensor.matmul(
                        ps2[:],
                        lhsT=w2[:, j * 128 : (j + 1) * 128],
                        rhs=xcbuf[:, jb * N : (jb + 1) * N],
                        start=(j == 0),
                        stop=(j == T - 1),
                    )

                for ch in range(NX):
                    r = route[ch % len(route)]
                    j0 = ch * CH_X
                    if r == "e":
                        for sub in range(CH_X // 2):
                            psb = psrep.tile([128, 2 * N], F32, tag="psrep")
                            for jj in range(2):
                                j = j0 + sub * 2 + jj
                                nc.tensor.matmul(
                                    psb[:, jj * N : (jj + 1) * N],
                                    lhsT=ident_mv[:, j : j + 1].to_broadcast(
                                        (128, 128)
                                    ),
                                    rhs=tde[:, :],
                                    start=True,
                                    stop=True,
                                    skip_group_check=True,
                                )
                            rte = rtpool.tile([128, 2 * N], MV, tag="rte")
                            nc.scalar.activation(
                                rte[:], psb[:], mybir.ActivationFunctionType.Identity
                            )
                            xce = xpool.tile([128, 2 * N], MV, tag="xce")
                            nc.vector.tensor_tensor(
                                out=xce[:].rearrange("p (c n) -> p c n", c=2),
                                in0=tok_cur[:, None, :].to_broadcast((128, 2, N)),
                                in1=rte[:].rearrange("p (c n) -> p c n", c=2),
                                op=mybir.AluOpType.mult,
                            )
                            for jj in range(2):
                                mm2(j0 + sub * 2 + jj, xce, jj)
                        continue
                    if r == "p":
                        rt = rtpool.tile([128, CH_X * N], MV, tag="rt")
                        for h in range(2):
                            hw = CH_X // 2 * N
                            row = rowpool.tile([1, hw], MV, tag="row")
                            nc.sync.dma_start(
                                row[:],
                                scr_flat[j0 * N + h * hw : j0 * N + (h + 1) * hw][
                                    None, :
                                ],
                            )
                            nc.gpsimd.partition_broadcast(
                                rt[:, h * hw : (h + 1) * hw], row[:]
                            )
                    else:
                        rt = rtpool.tile([128, CH_X * N], MV, tag="rt")
                        nc.sync.dma_start(
                            rt[:],
                            scr_flat[j0 * N : (j0 + CH_X) * N][None, :].to_broadcast(
                                (128, CH_X * N)
                            ),
                        )
                    xc = xpool.tile([128, CH_X * N], MV, tag="xc")
                    nc.vector.tensor_tensor(
                        out=xc[:].rearrange("p (c n) -> p c n", c=CH_X),
                        in0=tok_cur[:, None, :].to_broadcast((128, CH_X, N)),
                        in1=rt[:].rearrange("p (c n) -> p c n", c=CH_X),
                        op=mybir.AluOpType.mult,
                    )
                    for jl in range(CH_X):
                        mm2(j0 + jl, xc, jl)

                cnz = work.tile([128, N], F32, tag="cnz")
                nc.scalar.activation(
                    cnz[:],
                    ps2[:],
                    mybir.ActivationFunctionType.Tanh,
                    bias=b2c,
                    scale=1.0 / WSCALE,
                )
                last = blk == DEPTH - 1
                if not last:
                    delta = work.tile([128, N], F32, tag="delta")
                    nc.vector.tensor_scalar(
                        out=delta[:],
                        in0=cnz[:],
                        scalar1=cbg,
                        scalar2=None,
                        op0=mybir.AluOpType.subtract,
                    )
                else:
                    delta = cnz

                # ============ scatter (segment-sum over heads) ============
                if not last:
                    tok_next = work.tile([128, N], MV, tag="tokcur")
                for b in range(BL):
                    psT = pssm.tile([128, 128], F32, tag="pstr")
                    nc.tensor.transpose(
                        psT[:], delta[:, b * 128 : (b + 1) * 128], ident
                    )
                    dT = work.tile([128, 128], MV, tag="dT")
                    nc.vector.tensor_copy(dT[:], psT[:])
                    psS = pssm.tile([128, 128], F32, tag="psS")
                    hcol = (blk * BL + b) * 128
                    if not last:
                        nc.tensor.matmul(
                            psS[:],
                            lhsT=dT[:],
                            rhs=Hp[:, hcol : hcol + 128],
                            start=True,
                            stop=True,
                        )
                        nc.scalar.activation(
                            tok_next[:, b * 128 : (b + 1) * 128],
                            psS[:],
                            mybir.ActivationFunctionType.Identity,
                            bias=basec,
                        )
                    else:
                        nc.tensor.matmul(
                            psS[:],
                            lhsT=Hp[:, hcol : hcol + 128],
                            rhs=dT[:],
                            start=True,
                            stop=True,
                        )
                        osb = work.tile([128, 128], F32, tag="osb")
                        nc.vector.tensor_tensor(
                            out=osb[:],
                            in0=psS[:],
                            in1=baseB[:, b * 128 : (b + 1) * 128],
                            op=mybir.AluOpType.add,
                        )
                        nc.sync.dma_start(outh[b], osb[:])
                if not last:
                    tok_cur = tok_next
    nc.compile()
    return nc


def _get_program():
    """Return the most recently used program (for external profiling)."""
    if _PROG_CACHE:
        return next(reversed(_PROG_CACHE.values()))
    _PROG_CACHE[("dense",)] = build_program()
    return _PROG_CACHE[("dense",)]


def kernel_dense(token_table, dep_table, W1, b1, W2, b2, wr, br,
                 tokens, dep_types, dep_heads):
    global LAST_EXEC_TIME_NS
    W1f = W1.transpose(2, 1, 0).reshape(E * D, T)  # [(e,d), t]
    W1t = (np.ascontiguousarray(
        W1f.reshape(E * D // 128, 128, T).transpose(1, 0, 2).reshape(128, -1)
    ) * WSCALE).astype(ml_dtypes.float8_e4m3)
    W2f = W2.transpose(2, 1, 0).reshape(T * D, D)  # [(t,d), p]

    W2t = (np.ascontiguousarray(
        W2f.reshape(T * D // 128, 128, D).transpose(1, 0, 2).reshape(128, -1)
    ) * WSCALE).astype(ml_dtypes.float8_e4m3)
    c_bg = np.tanh(b2)
    base = (np.sum(wr) * c_bg + br).astype(np.float32)

    if ("dense",) not in _PROG_CACHE:
        _PROG_CACHE[("dense",)] = build_program()
    nc = _PROG_CACHE[("dense",)]
    in_maps = []
    for c in range(NCORES):
        bs = slice(c * BL, (c + 1) * BL)
        dep_c = dep_table[dep_types[bs]]  # [BL, S, E]
        dep_flat = np.ascontiguousarray(dep_c.reshape(N, E).T.reshape(1, E * N)).astype(
            ml_dtypes.bfloat16
        )
        tok0 = np.ascontiguousarray(
            token_table[tokens[bs]].reshape(N, D).T
        ).astype(ml_dtypes.bfloat16)
        heads_c = dep_heads[bs]
        mask_c = (heads_c == 0).astype(np.float32)  # [BL, S]
        cpack = np.zeros((128, C_TOT), dtype=np.float32)
        cpack[:, C_IDENT : C_IDENT + 128] = np.eye(128, dtype=np.float32)
        cpack[:, C_B1] = b1
        cpack[:, C_B2] = b2
        cpack[:, C_CBG] = c_bg
        cpack[:, C_BASE] = base
        for b in range(BL):
            colsum = np.bincount(heads_c[b], weights=wr, minlength=128)[:128]
            cpack[:, C_BASEB + b * 128 : C_BASEB + (b + 1) * 128] = mask_c[b][
                :, None
            ] * (base[None, :] - np.outer(colsum, c_bg))
        Hpack = np.zeros((128, DEPTH * BL * 128), dtype=np.float32)
        for blk in range(DEPTH):
            for b in range(BL):
                col = (blk * BL + b) * 128
                Hpack[np.arange(S), col + heads_c[b]] = wr
                if blk == DEPTH - 1:
                    Hpack[:, col : col + 128] *= mask_c[b][None, :]
        in_maps.append(
            {
                "W1t": W1t,
                "W2t": W2t,
                "cpack": cpack,
                "Hpack": np.ascontiguousarray(Hpack.astype(ml_dtypes.bfloat16)),
                "tok0": tok0,
                "dep_flat": dep_flat,
                "dep_cmp": np.ascontiguousarray(dep_flat.reshape(E, N)),
            }
        )

    trace = bool(int(os.environ.get("KERNEL_TRACE", "0")))
    res = run_bass_kernel_spmd(nc, in_maps, list(range(NCORES)), trace=trace)
    LAST_EXEC_TIME_NS = res.exec_time_ns
    out = np.concatenate([res.results[c]["out"] for c in range(NCORES)], axis=0)
    return np.ascontiguousarray(out.astype(np.float32))


# ---------------------------------------------------------------------------
# entry point
# ---------------------------------------------------------------------------

def kernel(
    token_table,
    dep_table,
    W1,
    b1,
    W2,
    b2,
    wr,
    br,
    tokens,
    dep_types,
    dep_heads,
):
    token_table = np.asarray(token_table, dtype=np.float32)
    dep_table = np.asarray(dep_table, dtype=np.float32)
    W1 = np.asarray(W1, dtype=np.float32)
    b1 = np.asarray(b1, dtype=np.float32)
    W2 = np.asarray(W2, dtype=np.float32)
    b2 = np.asarray(b2, dtype=np.float32)
    wr = np.asarray(wr, dtype=np.float32)
    br = np.asarray(br, dtype=np.float32)
    tokens = np.asarray(tokens).astype(np.int32)
    dep_types = np.asarray(dep_types).astype(np.int32)
    dep_heads = np.asarray(dep_heads).astype(np.int32)

    core_sets = [
        _sparse_sets(dep_heads[c * BL : (c + 1) * BL]) for c in range(NCORES)
    ]
    NA0 = _pad_cap(max(max(len(s[2]) for s in core_sets), 4))
    NA1 = _pad_cap(max(max(len(s[1]) for s in core_sets), 4))
    NR = _pad_cap(max(max(len(s[0]) for s in core_sets), 4))
    if NA0 <= 32 and NA1 <= 32 and NR <= 32:
        return kernel_sparse(
            token_table, dep_table, W1, b1, W2, b2, wr, br,
            tokens, dep_types, dep_heads, core_sets, NA0, NA1, NR,
        )
    return kernel_dense(
        token_table, dep_table, W1, b1, W2, b2, wr, br,
        tokens, dep_types, dep_heads,
    )
